# revision 46
# baseline (speedup 1.0000x reference)
"""MultiHeadAttention Bass kernel for TRN2, 8-core SPMD.

Sharding: core c -> batch b = c//4, heads [4*(c%4), 4*(c%4)+4).
Each core computes the qkv projection for its 4 heads, rope, attention,
and the out-projection partial (its 256 channels); host sums partials.

Device tensors (per core, host-prepped):
  xT        [1024, 2048]  bf16   x[b].T (channels on partitions)
  wqkT      [1024, 512]   bf16   q/k weight cols perm: [qA(128)|qB(128)|kA(128)|kB(128)]
                                 qA = even d-idx of 4 heads (4x32), qB = odds
  qkb       [128, 4]      f32    per-partition bias for the 4 o-tiles
  wvT       [1024, 256]   bf16   v weights, natural order
  vb_bc     [128, 256]    f32    v bias broadcast across partitions
  cos4/sin4 [128, 2048]   bf16   rope tables tiled 4x heads
  ind       [128, 128]    f32    row 64, cols 0:64 = 1 (recip broadcast matmul)
  projwT    [256, 1024]   bf16   out_w cols for this core's heads, transposed
  y         [1024, 2048]  f32    OUT: partial y^T (m on rows)

v2: scores via fp8e4 DoubleRow matmuls (K=64 even/odd packed as 2
k-subtiles of 32) -> half the score matmuls at 2x rate.  Rope outputs
written by DVE directly as fp8 into [128, 2, T] tiles.  QKV bias adds
on ACT (idle in phase A), normalization mul + y PSUM->SBUF copies on
Pool/GpSimd, un written in-place into the packed projection tile.
"""

import numpy as np
import ml_dtypes

import concourse.bass as bass
import concourse.tile as tile
from concourse import bacc, mybir
from concourse.bass import ts

F32 = mybir.dt.float32
BF16 = mybir.dt.bfloat16
FP8 = mybir.dt.float8e4
AF = mybir.ActivationFunctionType
DR = mybir.MatmulPerfMode.DoubleRow

B, T, DIM, NH = 2, 2048, 1024, 16
HD = 64          # head dim
HPC = 4          # heads per core
TC = 512         # t-chunk (one psum bank / fp32 matmul N limit)
TH = 1024        # t-half (exp op size)
NST = T // 128   # 16 s-tiles


def build(n_cores=8, loop_reps=1):
    nc = bacc.Bacc("TRN2", target_bir_lowering=False, debug=False,
                   num_devices=n_cores)

    xT_d = nc.dram_tensor("xT", [DIM, T], BF16, kind="ExternalInput").ap()
    x8_d = nc.dram_tensor("x8", [128, 8, T], FP8, kind="ExternalInput").ap()
    wqk8_d = nc.dram_tensor("wqk8", [128, 8, 512], FP8, kind="ExternalInput").ap()
    qkb_d = nc.dram_tensor("qkb", [128, 4], F32, kind="ExternalInput").ap()
    wvT_d = nc.dram_tensor("wvT", [DIM, 256], BF16, kind="ExternalInput").ap()
    vbbc_d = nc.dram_tensor("vb_bc", [128, 256], F32, kind="ExternalInput").ap()
    cos4_d = nc.dram_tensor("cos4", [128, T], BF16, kind="ExternalInput").ap()
    sin4_d = nc.dram_tensor("sin4", [128, T], BF16, kind="ExternalInput").ap()
    ind_d = nc.dram_tensor("ind", [128, 128], F32, kind="ExternalInput").ap()
    projwT_d = nc.dram_tensor("projwT", [256, 1024], BF16, kind="ExternalInput").ap()
    y_d = nc.dram_tensor("y", [DIM, T], BF16, kind="ExternalOutput").ap()

    with tile.TileContext(nc) as tc:
        if loop_reps > 1:
            with tc.For_i(0, loop_reps, 1):
                _kernel(nc, tc, xT_d, x8_d, wqk8_d, qkb_d, wvT_d, vbbc_d,
                        cos4_d, sin4_d, ind_d, projwT_d, y_d)
        else:
            _kernel(nc, tc, xT_d, x8_d, wqk8_d, qkb_d, wvT_d, vbbc_d,
                    cos4_d, sin4_d, ind_d, projwT_d, y_d)
    nc.compile()
    return nc


def _kernel(nc, tc, xT_d, x8_d, wqk8_d, qkb_d, wvT_d, vbbc_d, cos4_d, sin4_d,
            ind_d, projwT_d, y_d):
    from contextlib import ExitStack
    ctx = ExitStack()
    with ctx:
        # ---- constant / weight pools ----
        consts = ctx.enter_context(tc.tile_pool(name="consts", bufs=1))
        xpool = ctx.enter_context(tc.tile_pool(name="xp", bufs=1))

        # fp8 x / qk weights for the DoubleRow q/k projection go FIRST —
        # they gate the very first matmuls.  [128, j(8), cols], k-tile on dim 1
        wqk8 = consts.tile([128, 8, 512], FP8, tag="wqk8", name="wqk8")
        nc.sync.dma_start(wqk8[:], wqk8_d[:])
        x8 = xpool.tile([128, 8, T], FP8, tag="x8", name="x8")
        nc.sync.dma_start(x8[:], x8_d[:])
        qkb = consts.tile([128, 4], F32, tag="qkb")
        nc.sync.dma_start(qkb[:], qkb_d[:])
        cos4 = consts.tile([128, T], BF16, tag="cos4")
        nc.sync.dma_start(cos4[:], cos4_d[:])
        sin4 = consts.tile([128, T], BF16, tag="sin4")
        nc.sync.dma_start(sin4[:], sin4_d[:])
        xT = [xpool.tile([128, T], BF16, tag=f"xT{j}", name=f"xT{j}") for j in range(8)]
        for j in range(8):
            nc.sync.dma_start(xT[j][:], xT_d[ts(j, 128), :])
        wv = [consts.tile([128, 256], BF16, tag=f"wv{j}", name=f"wv{j}") for j in range(8)]
        for j in range(8):
            nc.sync.dma_start(wv[j][:], wvT_d[ts(j, 128), :])
        vbbc = consts.tile([128, 256], F32, tag="vbbc")
        nc.sync.dma_start(vbbc[:], vbbc_d[:])
        ind = consts.tile([128, 128], F32, tag="ind")
        nc.sync.dma_start(ind[:], ind_d[:])
        projw = [consts.tile([128, 1024], BF16, tag=f"pw{g}", name=f"pw{g}") for g in range(2)]
        for g in range(2):
            nc.sync.dma_start(projw[g][:], projwT_d[ts(g, 128), :])

        # ---- single shared PSUM pool: 4 tags x 2 banks = 8 banks ----
        ps = ctx.enter_context(tc.tile_pool(name="ps", bufs=1, space="PSUM"))

        def ps_tile(shape, tag):
            return ps.tile(shape, F32, tag=tag, name=f"ps_{tag}")

        # ---- phase A: QKV projection ----
        qksb = ctx.enter_context(tc.tile_pool(name="qksb", bufs=1))
        # o-tiles: 0=qA(evens) 1=qB(odds) 2=kA 3=kB
        qkt = [qksb.tile([128, T], BF16, tag=f"qk{o}", name=f"qk{o}") for o in range(4)]
        stags = ("sA", "sB")
        utags = ("uA", "uB")
        qk_pst = {}
        for o in range(4):
            for c in range(4):
                pst = ps_tile([128, TC], stags[(o * 4 + c) % 2])
                for kk in range(4):
                    nc.tensor.matmul(pst[:],
                                     wqk8[:, 2 * kk:2 * kk + 2, ts(o, 128)],
                                     x8[:, 2 * kk:2 * kk + 2, ts(c, TC)],
                                     start=(kk == 0), stop=(kk == 3),
                                     perf_mode=DR)
                qk_pst[(o, c)] = pst

        def qk_bias(o, c):
            # PSUM->SBUF move + bias on DVE, keeping ACT exp-only
            nc.vector.tensor_scalar_add(qkt[o][:, ts(c, TC)],
                                        qk_pst[(o, c)][:],
                                        qkb[:, o:o + 1])

        # v projection: [t, d'] layout, + ones column per head block
        vsb = ctx.enter_context(tc.tile_pool(name="vsb", bufs=1))
        vt = [vsb.tile([128, 260], BF16, tag=f"v{i}", name=f"v{i}") for i in range(NST)]
        v_pst = {}
        for i in range(NST):
            # v accumulates in the s-banks: the u-banks stay untouched by
            # phase A so the first pass's scores/exps never wait on v
            pst = ps_tile([128, 256], stags[i % 2])
            for j in range(8):
                nc.tensor.matmul(pst[:], xT[j][:, ts(i, 128)], wv[j][:],
                                 start=(j == 0), stop=(j == 7))
            v_pst[i] = pst

        def v_add(i):
            nc.vector.memset(vt[i][:], 1.0)
            # v block h at cols 65h:65h+64; col 65h+64 stays 1.0
            nc.vector.tensor_add(
                vt[i][:].rearrange("p (h d) -> p h d", h=4)[:, :, 0:64],
                v_pst[i][:].rearrange("p (h d) -> p h d", h=4),
                vbbc[:].rearrange("p (h d) -> p h d", h=4))

        # ---- rope: fp8 outputs in DoubleRow k-subtile layout ----
        # qEO/kEO [128, 2, T] fp8: [:,0,:] = rotated evens, [:,1,:] = odds
        ropet = ctx.enter_context(tc.tile_pool(name="ropet", bufs=2))
        rotsb = ctx.enter_context(tc.tile_pool(name="rotsb", bufs=1))
        qEO = rotsb.tile([128, 2, T], FP8, tag="qEO", name="qEO")
        kEO = rotsb.tile([128, 2, T], FP8, tag="kEO", name="kEO")

        def rope_chunk(which, c):
            a = qkt[0] if which == "q" else qkt[2]   # evens
            b = qkt[1] if which == "q" else qkt[3]   # odds
            eo = qEO if which == "q" else kEO
            cs = slice(c * TC, (c + 1) * TC)
            t1 = ropet.tile([128, TC], BF16, tag="t1")
            t2 = ropet.tile([128, TC], BF16, tag="t2")
            nc.vector.tensor_mul(t1[:], a[:, cs], cos4[:, cs])
            nc.vector.tensor_mul(t2[:], b[:, cs], sin4[:, cs])
            nc.vector.tensor_sub(eo[:, 0, cs], t1[:], t2[:])
            t3 = ropet.tile([128, TC], BF16, tag="t3")
            t4 = ropet.tile([128, TC], BF16, tag="t4")
            nc.vector.tensor_mul(t3[:], a[:, cs], sin4[:, cs])
            nc.vector.tensor_mul(t4[:], b[:, cs], cos4[:, cs])
            nc.vector.tensor_add(eo[:, 1, cs], t3[:], t4[:])

        # DVE order: rope chunks the first pass needs come first, v
        # bias-adds paced between the later chunks
        for oc in ((0, 0), (0, 1), (1, 0), (1, 1)):
            qk_bias(*oc)
        rope_chunk("q", 0)
        rope_chunk("q", 1)
        qk_bias(2, 0)
        qk_bias(3, 0)
        rope_chunk("k", 0)
        for i in (0, 1, 2, 3):
            v_add(i)
        qk_bias(2, 1)
        qk_bias(3, 1)
        rope_chunk("k", 1)
        for i in (4, 5, 6, 7):
            v_add(i)
        qk_bias(2, 2)
        qk_bias(3, 2)
        rope_chunk("k", 2)
        for i in (8, 9, 10, 11):
            v_add(i)
        qk_bias(2, 3)
        qk_bias(3, 3)
        rope_chunk("k", 3)
        for i in (12, 13, 14, 15):
            v_add(i)
        for oc in ((0, 2), (0, 3), (1, 2), (1, 3)):
            qk_bias(*oc)
        rope_chunk("q", 2)
        rope_chunk("q", 3)

        # ---- attention passes: (pair g, t-half th) ----
        ppool = ctx.enter_context(tc.tile_pool(name="pp", bufs=8))
        npool = ctx.enter_context(tc.tile_pool(name="np", bufs=2))
        unsb = ctx.enter_context(tc.tile_pool(name="unsb", bufs=1))
        ysb = ctx.enter_context(tc.tile_pool(name="ysb", bufs=3))

        # packed u_norm for the projection: [128 (pair c'), TH] per (g, th)
        upk = [[unsb.tile([128, TH], BF16, tag=f"upk{g}{th}", name=f"upk{g}{th}") for th in range(2)]
               for g in range(2)]

        def scores_exp(g, th, i):
            """Scores (fp8 DoubleRow, K=64 e/o packed) + exp for one s-tile;
            returns the two p tiles."""
            t0 = th * TH
            hA, hB = 2 * g, 2 * g + 1
            sps = {h: ps_tile([128, TH], utags[h - 2 * g])
                   for h in (hA, hB)}
            for c in range(2):
                for h in (hA, hB):
                    r = slice(32 * h, 32 * h + 32)
                    nc.tensor.matmul(
                        sps[h][:, ts(c, TC)],
                        kEO[r, :, ts(i, 128)],
                        qEO[r, :, t0 + c * TC:t0 + (c + 1) * TC],
                        start=True, stop=True, perf_mode=DR,
                        tile_position=(32 * h, 0))
            pd = {}
            for h in (hA, hB):
                p = ppool.tile([128, TH], BF16, tag=f"p{h - 2 * g}", name=f"p{h - 2 * g}")
                nc.scalar.activation(p[:], sps[h][:], AF.Exp, scale=0.125)
                pd[h] = p
            return pd

        def emit_pass(g, th, pre=None, nxt=None):
            """One (pair, t-half) attention pass.  `pre` carries p tiles
            prefetched by the previous pass; before our normalization chain
            we prefetch the first two score/exp groups of pass `nxt` so ACT
            never starves across the boundary."""
            hA, hB = 2 * g, 2 * g + 1
            u = {h: ps_tile([65, TH], stags[h - 2 * g]) for h in (hA, hB)}
            prefetched = None
            for i in range(NST):
                pd = pre[i] if (pre is not None and i < len(pre)) \
                    else scores_exp(g, th, i)
                for h in (hA, hB):
                    # AV + den: lhsT = [v_h | 1] (65 cols)
                    for c in range(2):
                        nc.tensor.matmul(
                            u[h][:, ts(c, TC)],
                            vt[i][:, 65 * h:65 * h + 65],
                            pd[h][:, ts(c, TC)],
                            start=(i == 0), stop=(i == NST - 1))
                if i == NST - 1 and nxt is not None:
                    prefetched = [scores_exp(nxt[0], nxt[1], 0),
                                  scores_exp(nxt[0], nxt[1], 1)]
            # normalization (per head): reciprocal of the denominator row,
            # ind-matmul broadcast to 64 partitions, then multiply — the
            # normalized attn out lands straight in the packed projection
            # tile (head A -> 0:64, B -> 64:128)
            for h in (hA, hB):
                dinv = npool.tile([65, TH], F32, tag="dinv", name="dinv")
                nc.vector.reciprocal(dinv[64:65, :], u[h][64:65, :])
                bc = ps_tile([128, TH], utags[h - 2 * g])
                for c in range(2):
                    nc.tensor.matmul(bc[:, ts(c, TC)],
                                     ind[64:65, :], dinv[64:65, ts(c, TC)],
                                     start=True, stop=True,
                                     tile_position=(64, 0))
                bcs = npool.tile([64, TH], F32, tag="bcs", name="bcs")
                nc.vector.tensor_copy(bcs[:], bc[0:64, :])
                nc.vector.tensor_mul(
                    upk[g][th][64 * (h % 2):64 * (h % 2) + 64, :],
                    u[h][0:64, :], bcs[:])
            return prefetched

        def proj_step(m, th, c, tag, idx):
            """One out-projection tile: yp = sum_g projw[g]^T @ upk[g][th].
            PSUM->SBUF copies alternate DVE/ACT, bf16 y DMAs alternate
            between the two hwdge queues."""
            yp = ps_tile([128, TC], tag)
            for g in range(2):
                nc.tensor.matmul(yp[:],
                                 projw[g][:, ts(m, 128)],
                                 upk[g][th][:, ts(c, TC)],
                                 start=(g == 0), stop=(g == 1))
            yt = ysb.tile([128, TC], BF16, tag="yt", name="yt")
            if idx % 2 == 0:
                nc.vector.tensor_copy(yt[:], yp[:])
            else:
                nc.scalar.copy(yt[:], yp[:])
            dma_eng = nc.sync if idx % 2 == 0 else nc.scalar
            dma_eng.dma_start(
                y_d[ts(m, 128), th * TH + c * TC:th * TH + (c + 1) * TC],
                yt[:])

        order = [(0, 0), (0, 1), (1, 0), (1, 1)]
        pre = None
        for n, (g, th) in enumerate(order):
            nxt = order[n + 1] if n + 1 < len(order) else None
            pre = emit_pass(g, th, pre=pre, nxt=nxt)
        for idx, (pm, pth, pc) in enumerate(
                [(m, th, c) for m in range(8) for th in range(2)
                 for c in range(2)]):
            proj_step(pm, pth, pc, stags[idx % 2], idx)


# ---------------- host-side prep / gather ----------------

def rope_tables():
    hd = HD
    inv_freq = 1.0 / (10000.0 ** (np.arange(0, hd, 2, dtype=np.float32) / hd))
    t = np.arange(T, dtype=np.float32)
    freqs = t[:, None] * inv_freq[None, :]                  # [T, 32]
    emb = np.concatenate([np.sin(freqs), np.cos(freqs)], axis=-1)  # [T,64]
    sin_t = emb[:, 0::2].T.astype(np.float32)               # [32, T]
    cos_t = emb[:, 1::2].T.astype(np.float32)
    return sin_t, cos_t


def make_in_maps(x, qkv_w, qkv_b, out_w):
    """Returns list of 8 per-core input dicts."""
    bf = ml_dtypes.bfloat16
    sin_t, cos_t = rope_tables()
    cos4 = np.tile(cos_t, (4, 1)).astype(bf)
    sin4 = np.tile(sin_t, (4, 1)).astype(bf)
    ind = np.zeros((128, 128), np.float32)
    ind[64, 0:64] = 1.0
    ev = np.arange(0, HD, 2)
    od = np.arange(1, HD, 2)

    in_maps = []
    for core in range(8):
        b = core // 4
        h0 = HPC * (core % 4)
        heads = np.arange(h0, h0 + HPC)
        qA = np.concatenate([h * HD + ev for h in heads])          # 128
        qB = np.concatenate([h * HD + od for h in heads])
        kA = DIM + qA
        kB = DIM + qB
        qk_rows = np.concatenate([qA, qB, kA, kB])                  # 512
        v_rows = 2 * DIM + np.arange(h0 * HD, (h0 + HPC) * HD)      # 256
        f8 = ml_dtypes.float8_e4m3
        wqk8 = np.ascontiguousarray(
            qkv_w[qk_rows, :].T.reshape(8, 128, 512).transpose(1, 0, 2)
        ).astype(f8)                                                 # [128,8,512]
        qkb = np.ascontiguousarray(
            qkv_b[qk_rows].reshape(4, 128).T).astype(np.float32)     # [128,4]
        wvT = np.ascontiguousarray(qkv_w[v_rows, :].T).astype(bf)    # [1024,256]
        vb_bc = np.broadcast_to(qkv_b[v_rows], (128, 256)).astype(np.float32)
        projwT = np.ascontiguousarray(
            out_w[:, h0 * HD:(h0 + HPC) * HD].T).astype(bf)          # [256,1024]
        xT = np.ascontiguousarray(x[b].T).astype(bf)                 # [1024,2048]
        x8 = np.ascontiguousarray(
            x[b].T.reshape(8, 128, T).transpose(1, 0, 2)).astype(f8)  # [128,8,T]
        in_maps.append({
            "xT": np.asarray(xT), "x8": np.asarray(x8),
            "wqk8": np.asarray(wqk8), "qkb": qkb,
            "wvT": np.asarray(wvT), "vb_bc": np.ascontiguousarray(vb_bc),
            "cos4": np.asarray(cos4), "sin4": np.asarray(sin4),
            "ind": ind, "projwT": np.asarray(projwT),
        })
    return in_maps


def gather(results, out_b):
    """results: list of 8 dicts with y [1024, 2048] f32 partials."""
    y = np.zeros((B, T, DIM), np.float32)
    for core in range(8):
        b = core // 4
        y[b] += results[core]["y"].T.astype(np.float32)
    y += out_b[None, None, :]
    return y


# ---------------- harness entry point ----------------

_NC_CACHE = {}


def kernel(x, qkv_w, qkv_b, out_w, out_b):
    """Full-input entry: shards across 8 NeuronCores, returns full output."""
    from concourse import bass_utils
    x = np.asarray(x); qkv_w = np.asarray(qkv_w); qkv_b = np.asarray(qkv_b)
    out_w = np.asarray(out_w); out_b = np.asarray(out_b)
    if "nc" not in _NC_CACHE:
        _NC_CACHE["nc"] = build(n_cores=8)
    nc = _NC_CACHE["nc"]
    in_maps = make_in_maps(x, qkv_w, qkv_b, out_w)
    res = bass_utils.run_bass_kernel_spmd(nc, in_maps, core_ids=list(range(8)))
    return gather(res.results, out_b)


# revision 47
# speedup vs baseline: 1.0595x; 1.0595x over previous
"""MultiHeadAttention Bass kernel for TRN2, 8-core SPMD.

Sharding: core c -> batch b = c//4, heads [4*(c%4), 4*(c%4)+4).
Each core computes the qkv projection for its 4 heads, rope, attention,
and the out-projection partial (its 256 channels); host sums partials.

Device tensors (per core, host-prepped):
  xT        [1024, 2048]  bf16   x[b].T (channels on partitions)
  wqkT      [1024, 512]   bf16   q/k weight cols perm: [qA(128)|qB(128)|kA(128)|kB(128)]
                                 qA = even d-idx of 4 heads (4x32), qB = odds
  qkb       [128, 4]      f32    per-partition bias for the 4 o-tiles
  wvT       [1024, 256]   bf16   v weights, natural order
  vb_bc     [128, 256]    f32    v bias broadcast across partitions
  cos4/sin4 [128, 2048]   bf16   rope tables tiled 4x heads
  ind       [128, 128]    f32    row 64, cols 0:64 = 1 (recip broadcast matmul)
  projwT    [256, 1024]   bf16   out_w cols for this core's heads, transposed
  y         [1024, 2048]  f32    OUT: partial y^T (m on rows)

v2: scores via fp8e4 DoubleRow matmuls (K=64 even/odd packed as 2
k-subtiles of 32) -> half the score matmuls at 2x rate.  Rope outputs
written by DVE directly as fp8 into [128, 2, T] tiles.  QKV bias adds
on ACT (idle in phase A), normalization mul + y PSUM->SBUF copies on
Pool/GpSimd, un written in-place into the packed projection tile.
"""

import numpy as np
import ml_dtypes

import concourse.bass as bass
import concourse.tile as tile
from concourse import bacc, mybir
from concourse.bass import ts

F32 = mybir.dt.float32
BF16 = mybir.dt.bfloat16
FP8 = mybir.dt.float8e4
AF = mybir.ActivationFunctionType
DR = mybir.MatmulPerfMode.DoubleRow

B, T, DIM, NH = 2, 2048, 1024, 16
HD = 64          # head dim
HPC = 4          # heads per core
TC = 512         # t-chunk (one psum bank / fp32 matmul N limit)
TH = 1024        # t-half (exp op size)
NST = T // 128   # 16 s-tiles


def build(n_cores=8, loop_reps=1):
    nc = bacc.Bacc("TRN2", target_bir_lowering=False, debug=False,
                   num_devices=n_cores)

    xT_d = nc.dram_tensor("xT", [DIM, T], BF16, kind="ExternalInput").ap()
    x8_d = nc.dram_tensor("x8", [128, 8, T], FP8, kind="ExternalInput").ap()
    wqk8_d = nc.dram_tensor("wqk8", [128, 8, 512], FP8, kind="ExternalInput").ap()
    qkb_d = nc.dram_tensor("qkb", [128, 4], F32, kind="ExternalInput").ap()
    wvT_d = nc.dram_tensor("wvT", [DIM, 256], BF16, kind="ExternalInput").ap()
    vbbc_d = nc.dram_tensor("vb_bc", [128, 256], F32, kind="ExternalInput").ap()
    cos4_d = nc.dram_tensor("cos4", [128, T], BF16, kind="ExternalInput").ap()
    sin4_d = nc.dram_tensor("sin4", [128, T], BF16, kind="ExternalInput").ap()
    ind_d = nc.dram_tensor("ind", [128, 128], F32, kind="ExternalInput").ap()
    projwT_d = nc.dram_tensor("projwT", [256, 1024], BF16, kind="ExternalInput").ap()
    y_d = nc.dram_tensor("y", [DIM, T], BF16, kind="ExternalOutput").ap()

    with tile.TileContext(nc) as tc:
        if loop_reps > 1:
            with tc.For_i(0, loop_reps, 1):
                _kernel(nc, tc, xT_d, x8_d, wqk8_d, qkb_d, wvT_d, vbbc_d,
                        cos4_d, sin4_d, ind_d, projwT_d, y_d)
        else:
            _kernel(nc, tc, xT_d, x8_d, wqk8_d, qkb_d, wvT_d, vbbc_d,
                    cos4_d, sin4_d, ind_d, projwT_d, y_d)
    nc.compile()
    return nc


def _kernel(nc, tc, xT_d, x8_d, wqk8_d, qkb_d, wvT_d, vbbc_d, cos4_d, sin4_d,
            ind_d, projwT_d, y_d):
    from contextlib import ExitStack
    ctx = ExitStack()
    with ctx:
        # ---- constant / weight pools ----
        consts = ctx.enter_context(tc.tile_pool(name="consts", bufs=1))
        xpool = ctx.enter_context(tc.tile_pool(name="xp", bufs=1))

        # fp8 x / qk weights for the DoubleRow q/k projection go FIRST —
        # they gate the very first matmuls.  [128, j(8), cols], k-tile on dim 1
        wqk8 = consts.tile([128, 8, 512], FP8, tag="wqk8", name="wqk8")
        nc.sync.dma_start(wqk8[:], wqk8_d[:])
        x8 = xpool.tile([128, 8, T], FP8, tag="x8", name="x8")
        nc.sync.dma_start(x8[:], x8_d[:])
        qkb = consts.tile([128, 4], F32, tag="qkb")
        nc.sync.dma_start(qkb[:], qkb_d[:])
        cos4 = consts.tile([128, T], BF16, tag="cos4")
        nc.sync.dma_start(cos4[:], cos4_d[:])
        sin4 = consts.tile([128, T], BF16, tag="sin4")
        nc.sync.dma_start(sin4[:], sin4_d[:])
        xT = [xpool.tile([128, T], BF16, tag=f"xT{j}", name=f"xT{j}") for j in range(8)]
        for j in range(8):
            nc.sync.dma_start(xT[j][:], xT_d[ts(j, 128), :])
        wv = [consts.tile([128, 256], BF16, tag=f"wv{j}", name=f"wv{j}") for j in range(8)]
        for j in range(8):
            nc.sync.dma_start(wv[j][:], wvT_d[ts(j, 128), :])
        vbbc = consts.tile([128, 256], F32, tag="vbbc")
        nc.sync.dma_start(vbbc[:], vbbc_d[:])
        ind = consts.tile([128, 128], F32, tag="ind")
        nc.sync.dma_start(ind[:], ind_d[:])
        projw = [consts.tile([128, 1024], BF16, tag=f"pw{g}", name=f"pw{g}") for g in range(2)]
        for g in range(2):
            nc.sync.dma_start(projw[g][:], projwT_d[ts(g, 128), :])

        # ---- single shared PSUM pool: 4 tags x 2 banks = 8 banks ----
        ps = ctx.enter_context(tc.tile_pool(name="ps", bufs=1, space="PSUM"))

        def ps_tile(shape, tag):
            return ps.tile(shape, F32, tag=tag, name=f"ps_{tag}")

        # ---- phase A: QKV projection ----
        qksb = ctx.enter_context(tc.tile_pool(name="qksb", bufs=1))
        # o-tiles: 0=qA(evens) 1=qB(odds) 2=kA 3=kB
        qkt = [qksb.tile([128, T], BF16, tag=f"qk{o}", name=f"qk{o}") for o in range(4)]
        stags = ("sA", "sB")
        utags = ("uA", "uB")
        for o in range(4):
            for c in range(4):
                pst = ps_tile([128, TC], stags[(o * 4 + c) % 2])
                for kk in range(4):
                    nc.tensor.matmul(pst[:],
                                     wqk8[:, 2 * kk:2 * kk + 2, ts(o, 128)],
                                     x8[:, 2 * kk:2 * kk + 2, ts(c, TC)],
                                     start=(kk == 0), stop=(kk == 3),
                                     perf_mode=DR)
                # bias add on ACT (idle during phase A); DVE is busy w/ rope
                nc.scalar.activation(qkt[o][:, ts(c, TC)], pst[:],
                                     AF.Identity, bias=qkb[:, o:o + 1])

        # v projection: [t, d'] layout, + ones column per head block
        vsb = ctx.enter_context(tc.tile_pool(name="vsb", bufs=1))
        vt = [vsb.tile([128, 260], BF16, tag=f"v{i}", name=f"v{i}") for i in range(NST)]
        v_pst = {}
        for i in range(NST):
            # v accumulates in the s-banks: the u-banks stay untouched by
            # phase A so the first pass's scores/exps never wait on v
            pst = ps_tile([128, 256], stags[i % 2])
            for j in range(8):
                nc.tensor.matmul(pst[:], xT[j][:, ts(i, 128)], wv[j][:],
                                 start=(j == 0), stop=(j == 7))
            v_pst[i] = pst

        def v_add(i):
            nc.vector.memset(vt[i][:], 1.0)
            # v block h at cols 65h:65h+64; col 65h+64 stays 1.0
            nc.vector.tensor_add(
                vt[i][:].rearrange("p (h d) -> p h d", h=4)[:, :, 0:64],
                v_pst[i][:].rearrange("p (h d) -> p h d", h=4),
                vbbc[:].rearrange("p (h d) -> p h d", h=4))

        # ---- rope: fp8 outputs in DoubleRow k-subtile layout ----
        # qEO/kEO [128, 2, T] fp8: [:,0,:] = rotated evens, [:,1,:] = odds
        ropet = ctx.enter_context(tc.tile_pool(name="ropet", bufs=2))
        rotsb = ctx.enter_context(tc.tile_pool(name="rotsb", bufs=1))
        qEO = rotsb.tile([128, 2, T], FP8, tag="qEO", name="qEO")
        kEO = rotsb.tile([128, 2, T], FP8, tag="kEO", name="kEO")

        def rope_chunk(which, c):
            a = qkt[0] if which == "q" else qkt[2]   # evens
            b = qkt[1] if which == "q" else qkt[3]   # odds
            eo = qEO if which == "q" else kEO
            cs = slice(c * TC, (c + 1) * TC)
            t1 = ropet.tile([128, TC], BF16, tag="t1")
            t2 = ropet.tile([128, TC], BF16, tag="t2")
            nc.vector.tensor_mul(t1[:], a[:, cs], cos4[:, cs])
            nc.vector.tensor_mul(t2[:], b[:, cs], sin4[:, cs])
            nc.vector.tensor_sub(eo[:, 0, cs], t1[:], t2[:])
            t3 = ropet.tile([128, TC], BF16, tag="t3")
            t4 = ropet.tile([128, TC], BF16, tag="t4")
            nc.vector.tensor_mul(t3[:], a[:, cs], sin4[:, cs])
            nc.vector.tensor_mul(t4[:], b[:, cs], cos4[:, cs])
            nc.vector.tensor_add(eo[:, 1, cs], t3[:], t4[:])

        # DVE order: rope chunks the first pass needs come first, v
        # bias-adds paced between the later chunks
        rope_chunk("q", 0)
        rope_chunk("q", 1)
        rope_chunk("k", 0)
        for i in (0, 1, 2, 3):
            v_add(i)
        rope_chunk("k", 1)
        for i in (4, 5, 6, 7):
            v_add(i)
        rope_chunk("k", 2)
        for i in (8, 9, 10, 11):
            v_add(i)
        rope_chunk("k", 3)
        for i in (12, 13, 14, 15):
            v_add(i)
        rope_chunk("q", 2)
        rope_chunk("q", 3)

        # ---- attention passes: (pair g, t-half th) ----
        ppool = ctx.enter_context(tc.tile_pool(name="pp", bufs=8))
        npool = ctx.enter_context(tc.tile_pool(name="np", bufs=2))
        unsb = ctx.enter_context(tc.tile_pool(name="unsb", bufs=1))
        ysb = ctx.enter_context(tc.tile_pool(name="ysb", bufs=3))

        # packed u_norm for the projection: [128 (pair c'), TH] per (g, th)
        upk = [[unsb.tile([128, TH], BF16, tag=f"upk{g}{th}", name=f"upk{g}{th}") for th in range(2)]
               for g in range(2)]

        def scores_exp(g, th, i):
            """Scores (fp8 DoubleRow, K=64 e/o packed) + exp for one s-tile;
            returns the two p tiles."""
            t0 = th * TH
            hA, hB = 2 * g, 2 * g + 1
            sps = {h: ps_tile([128, TH], utags[h - 2 * g])
                   for h in (hA, hB)}
            for c in range(2):
                for h in (hA, hB):
                    r = slice(32 * h, 32 * h + 32)
                    nc.tensor.matmul(
                        sps[h][:, ts(c, TC)],
                        kEO[r, :, ts(i, 128)],
                        qEO[r, :, t0 + c * TC:t0 + (c + 1) * TC],
                        start=True, stop=True, perf_mode=DR,
                        tile_position=(32 * h, 0))
            pd = {}
            for h in (hA, hB):
                p = ppool.tile([128, TH], BF16, tag=f"p{h - 2 * g}", name=f"p{h - 2 * g}")
                nc.scalar.activation(p[:], sps[h][:], AF.Exp, scale=0.125)
                pd[h] = p
            return pd

        def emit_pass(g, th, pre=None, nxt=None):
            """One (pair, t-half) attention pass.  `pre` carries p tiles
            prefetched by the previous pass; before our normalization chain
            we prefetch the first two score/exp groups of pass `nxt` so ACT
            never starves across the boundary."""
            hA, hB = 2 * g, 2 * g + 1
            u = {h: ps_tile([65, TH], stags[h - 2 * g]) for h in (hA, hB)}
            prefetched = None
            for i in range(NST):
                pd = pre[i] if (pre is not None and i < len(pre)) \
                    else scores_exp(g, th, i)
                for h in (hA, hB):
                    # AV + den: lhsT = [v_h | 1] (65 cols)
                    for c in range(2):
                        nc.tensor.matmul(
                            u[h][:, ts(c, TC)],
                            vt[i][:, 65 * h:65 * h + 65],
                            pd[h][:, ts(c, TC)],
                            start=(i == 0), stop=(i == NST - 1))
                if i == NST - 1 and nxt is not None:
                    prefetched = [scores_exp(nxt[0], nxt[1], 0),
                                  scores_exp(nxt[0], nxt[1], 1)]
            # normalization (per head): reciprocal of the denominator row,
            # ind-matmul broadcast to 64 partitions, then multiply — the
            # normalized attn out lands straight in the packed projection
            # tile (head A -> 0:64, B -> 64:128)
            for h in (hA, hB):
                dinv = npool.tile([65, TH], F32, tag="dinv", name="dinv")
                nc.vector.reciprocal(dinv[64:65, :], u[h][64:65, :])
                bc = ps_tile([128, TH], utags[h - 2 * g])
                for c in range(2):
                    nc.tensor.matmul(bc[:, ts(c, TC)],
                                     ind[64:65, :], dinv[64:65, ts(c, TC)],
                                     start=True, stop=True,
                                     tile_position=(64, 0))
                bcs = npool.tile([64, TH], F32, tag="bcs", name="bcs")
                nc.vector.tensor_copy(bcs[:], bc[0:64, :])
                nc.vector.tensor_mul(
                    upk[g][th][64 * (h % 2):64 * (h % 2) + 64, :],
                    u[h][0:64, :], bcs[:])
            return prefetched

        def proj_step(m, th, c, tag, idx):
            """One out-projection tile: yp = sum_g projw[g]^T @ upk[g][th].
            PSUM->SBUF copies alternate DVE/ACT, bf16 y DMAs alternate
            between the two hwdge queues."""
            yp = ps_tile([128, TC], tag)
            for g in range(2):
                nc.tensor.matmul(yp[:],
                                 projw[g][:, ts(m, 128)],
                                 upk[g][th][:, ts(c, TC)],
                                 start=(g == 0), stop=(g == 1))
            yt = ysb.tile([128, TC], BF16, tag="yt", name="yt")
            if idx % 2 == 0:
                nc.vector.tensor_copy(yt[:], yp[:])
            else:
                nc.scalar.copy(yt[:], yp[:])
            dma_eng = nc.sync if idx % 2 == 0 else nc.scalar
            dma_eng.dma_start(
                y_d[ts(m, 128), th * TH + c * TC:th * TH + (c + 1) * TC],
                yt[:])

        order = [(0, 0), (0, 1), (1, 0), (1, 1)]
        pre = None
        for n, (g, th) in enumerate(order):
            nxt = order[n + 1] if n + 1 < len(order) else None
            pre = emit_pass(g, th, pre=pre, nxt=nxt)
        for idx, (pm, pth, pc) in enumerate(
                [(m, th, c) for m in range(8) for th in range(2)
                 for c in range(2)]):
            proj_step(pm, pth, pc, stags[idx % 2], idx)


# ---------------- host-side prep / gather ----------------

def rope_tables():
    hd = HD
    inv_freq = 1.0 / (10000.0 ** (np.arange(0, hd, 2, dtype=np.float32) / hd))
    t = np.arange(T, dtype=np.float32)
    freqs = t[:, None] * inv_freq[None, :]                  # [T, 32]
    emb = np.concatenate([np.sin(freqs), np.cos(freqs)], axis=-1)  # [T,64]
    sin_t = emb[:, 0::2].T.astype(np.float32)               # [32, T]
    cos_t = emb[:, 1::2].T.astype(np.float32)
    return sin_t, cos_t


def make_in_maps(x, qkv_w, qkv_b, out_w):
    """Returns list of 8 per-core input dicts."""
    bf = ml_dtypes.bfloat16
    sin_t, cos_t = rope_tables()
    cos4 = np.tile(cos_t, (4, 1)).astype(bf)
    sin4 = np.tile(sin_t, (4, 1)).astype(bf)
    ind = np.zeros((128, 128), np.float32)
    ind[64, 0:64] = 1.0
    ev = np.arange(0, HD, 2)
    od = np.arange(1, HD, 2)

    in_maps = []
    for core in range(8):
        b = core // 4
        h0 = HPC * (core % 4)
        heads = np.arange(h0, h0 + HPC)
        qA = np.concatenate([h * HD + ev for h in heads])          # 128
        qB = np.concatenate([h * HD + od for h in heads])
        kA = DIM + qA
        kB = DIM + qB
        qk_rows = np.concatenate([qA, qB, kA, kB])                  # 512
        v_rows = 2 * DIM + np.arange(h0 * HD, (h0 + HPC) * HD)      # 256
        f8 = ml_dtypes.float8_e4m3
        wqk8 = np.ascontiguousarray(
            qkv_w[qk_rows, :].T.reshape(8, 128, 512).transpose(1, 0, 2)
        ).astype(f8)                                                 # [128,8,512]
        qkb = np.ascontiguousarray(
            qkv_b[qk_rows].reshape(4, 128).T).astype(np.float32)     # [128,4]
        wvT = np.ascontiguousarray(qkv_w[v_rows, :].T).astype(bf)    # [1024,256]
        vb_bc = np.broadcast_to(qkv_b[v_rows], (128, 256)).astype(np.float32)
        projwT = np.ascontiguousarray(
            out_w[:, h0 * HD:(h0 + HPC) * HD].T).astype(bf)          # [256,1024]
        xT = np.ascontiguousarray(x[b].T).astype(bf)                 # [1024,2048]
        x8 = np.ascontiguousarray(
            x[b].T.reshape(8, 128, T).transpose(1, 0, 2)).astype(f8)  # [128,8,T]
        in_maps.append({
            "xT": np.asarray(xT), "x8": np.asarray(x8),
            "wqk8": np.asarray(wqk8), "qkb": qkb,
            "wvT": np.asarray(wvT), "vb_bc": np.ascontiguousarray(vb_bc),
            "cos4": np.asarray(cos4), "sin4": np.asarray(sin4),
            "ind": ind, "projwT": np.asarray(projwT),
        })
    return in_maps


def gather(results, out_b):
    """results: list of 8 dicts with y [1024, 2048] f32 partials."""
    y = np.zeros((B, T, DIM), np.float32)
    for core in range(8):
        b = core // 4
        y[b] += results[core]["y"].T.astype(np.float32)
    y += out_b[None, None, :]
    return y


# ---------------- harness entry point ----------------

_NC_CACHE = {}


def kernel(x, qkv_w, qkv_b, out_w, out_b):
    """Full-input entry: shards across 8 NeuronCores, returns full output."""
    from concourse import bass_utils
    x = np.asarray(x); qkv_w = np.asarray(qkv_w); qkv_b = np.asarray(qkv_b)
    out_w = np.asarray(out_w); out_b = np.asarray(out_b)
    if "nc" not in _NC_CACHE:
        _NC_CACHE["nc"] = build(n_cores=8)
    nc = _NC_CACHE["nc"]
    in_maps = make_in_maps(x, qkv_w, qkv_b, out_w)
    res = bass_utils.run_bass_kernel_spmd(nc, in_maps, core_ids=list(range(8)))
    return gather(res.results, out_b)


# revision 48
# speedup vs baseline: 1.1769x; 1.1108x over previous
"""MultiHeadAttention Bass kernel for TRN2, 8-core SPMD.

Sharding: core c -> batch b = c//4, heads [4*(c%4), 4*(c%4)+4).
Each core computes the qkv projection for its 4 heads, rope, attention,
and the out-projection partial (its 256 channels); host sums partials.

Device tensors (per core, host-prepped):
  xT        [1024, 2048]  bf16   x[b].T (channels on partitions; v-proj)
  x8        [128, 8, 2048] fp8e4 x[b].T as 8 k-tiles on dim 1 (q/k-proj)
  wqk8      [128, 8, 512] fp8e4  q/k weight cols perm [qA|qB|kA|kB],
                                 qA = even d-idx of 4 heads (4x32), qB = odds
  qkb       [128, 4]      f32    per-partition bias for the 4 o-tiles
  wvT       [1024, 256]   bf16   v weights, natural order
  vb_bc     [128, 256]    f32    v bias broadcast across partitions
  cos4/sin4 [128, 2048]   bf16   rope tables tiled 4x heads
  ind       [128, 128]    f32    row 64, cols 0:64 = 1 (recip broadcast matmul)
  projwT    [256, 1024]   bf16   out_w cols for this core's heads, transposed
  y         [1024, 2048]  bf16   OUT: partial y^T (m on rows); host upcasts

Key performance structure (For_i repeats run back-to-back behind an
all-engine barrier, so the single-iteration critical path is what counts):
 - q/k projection and scores run as fp8e4 DoubleRow matmuls (2 k-subtiles
   per pass, 0.5 cyc/col); scores pack K=64 = even|odd rope halves from
   [128, 2, T] fp8 rope outputs written directly by DVE.
 - ACT is the bottleneck engine (128 exps of [128,1024]); everything else
   is scheduled around keeping it fed: critical DMAs (wqk8, x8, rope
   tables) issue first; rope is emitted in 512-col chunks interleaved
   with the v bias-adds so the first exp fires ~12us in; each pass
   prefetches the next pass's first two score/exp groups before its own
   normalization chain so ACT never starves at pass boundaries.
 - PSUM tag roles: phase A + u-accumulators on sA/sB, scores/bc on uA/uB
   (so the exp stream never waits on the v pipeline); deep p-pool lets
   AV lag behind exp.
 - tail: out-projection drains on the s-banks; PSUM->SBUF copies
   alternate DVE/ACT and bf16 y DMAs alternate across both hwdge queues.
"""

import numpy as np
import ml_dtypes

import concourse.bass as bass
import concourse.tile as tile
from concourse import bacc, mybir
from concourse.bass import ts

F32 = mybir.dt.float32
BF16 = mybir.dt.bfloat16
FP8 = mybir.dt.float8e4
AF = mybir.ActivationFunctionType
DR = mybir.MatmulPerfMode.DoubleRow

B, T, DIM, NH = 2, 2048, 1024, 16
HD = 64          # head dim
HPC = 4          # heads per core
TC = 512         # t-chunk (one psum bank / fp32 matmul N limit)
TH = 1024        # t-half (exp op size)
NST = T // 128   # 16 s-tiles


def build(n_cores=8, loop_reps=1):
    nc = bacc.Bacc("TRN2", target_bir_lowering=False, debug=False,
                   num_devices=n_cores)

    xT_d = nc.dram_tensor("xT", [DIM, T], BF16, kind="ExternalInput").ap()
    x8_d = nc.dram_tensor("x8", [128, 8, T], FP8, kind="ExternalInput").ap()
    wqk8_d = nc.dram_tensor("wqk8", [128, 8, 512], FP8, kind="ExternalInput").ap()
    qkb_d = nc.dram_tensor("qkb", [128, 4], F32, kind="ExternalInput").ap()
    wvT_d = nc.dram_tensor("wvT", [DIM, 256], BF16, kind="ExternalInput").ap()
    vbbc_d = nc.dram_tensor("vb_bc", [128, 256], F32, kind="ExternalInput").ap()
    cos4_d = nc.dram_tensor("cos4", [128, T], BF16, kind="ExternalInput").ap()
    sin4_d = nc.dram_tensor("sin4", [128, T], BF16, kind="ExternalInput").ap()
    ind_d = nc.dram_tensor("ind", [128, 128], F32, kind="ExternalInput").ap()
    projwT_d = nc.dram_tensor("projwT", [256, 1024], BF16, kind="ExternalInput").ap()
    y_d = nc.dram_tensor("y", [DIM, T], BF16, kind="ExternalOutput").ap()

    with tile.TileContext(nc) as tc:
        if loop_reps > 1:
            with tc.For_i(0, loop_reps, 1):
                _kernel(nc, tc, xT_d, x8_d, wqk8_d, qkb_d, wvT_d, vbbc_d,
                        cos4_d, sin4_d, ind_d, projwT_d, y_d)
        else:
            _kernel(nc, tc, xT_d, x8_d, wqk8_d, qkb_d, wvT_d, vbbc_d,
                    cos4_d, sin4_d, ind_d, projwT_d, y_d)
    nc.compile()
    return nc


def _kernel(nc, tc, xT_d, x8_d, wqk8_d, qkb_d, wvT_d, vbbc_d, cos4_d, sin4_d,
            ind_d, projwT_d, y_d):
    from contextlib import ExitStack
    ctx = ExitStack()
    with ctx:
        # ---- constant / weight pools ----
        consts = ctx.enter_context(tc.tile_pool(name="consts", bufs=1))
        xpool = ctx.enter_context(tc.tile_pool(name="xp", bufs=1))

        # fp8 x / qk weights for the DoubleRow q/k projection go FIRST —
        # they gate the very first matmuls.  [128, j(8), cols], k-tile on dim 1
        wqk8 = consts.tile([128, 8, 512], FP8, tag="wqk8", name="wqk8")
        nc.sync.dma_start(wqk8[:], wqk8_d[:])
        x8 = xpool.tile([128, 8, T], FP8, tag="x8", name="x8")
        nc.sync.dma_start(x8[:], x8_d[:])
        qkb = consts.tile([128, 4], F32, tag="qkb")
        nc.sync.dma_start(qkb[:], qkb_d[:])
        cos4 = consts.tile([128, T], BF16, tag="cos4")
        nc.sync.dma_start(cos4[:], cos4_d[:])
        sin4 = consts.tile([128, T], BF16, tag="sin4")
        nc.sync.dma_start(sin4[:], sin4_d[:])
        xT = [xpool.tile([128, T], BF16, tag=f"xT{j}", name=f"xT{j}") for j in range(8)]
        for j in range(8):
            nc.sync.dma_start(xT[j][:], xT_d[ts(j, 128), :])
        wv = [consts.tile([128, 256], BF16, tag=f"wv{j}", name=f"wv{j}") for j in range(8)]
        for j in range(8):
            nc.sync.dma_start(wv[j][:], wvT_d[ts(j, 128), :])
        vbbc = consts.tile([128, 256], F32, tag="vbbc")
        nc.sync.dma_start(vbbc[:], vbbc_d[:])
        ind = consts.tile([128, 128], F32, tag="ind")
        nc.sync.dma_start(ind[:], ind_d[:])
        projw = [consts.tile([128, 1024], BF16, tag=f"pw{g}", name=f"pw{g}") for g in range(2)]
        for g in range(2):
            nc.sync.dma_start(projw[g][:], projwT_d[ts(g, 128), :])

        # ---- single shared PSUM pool: 4 tags x 2 banks = 8 banks ----
        ps = ctx.enter_context(tc.tile_pool(name="ps", bufs=1, space="PSUM"))

        def ps_tile(shape, tag):
            return ps.tile(shape, F32, tag=tag, name=f"ps_{tag}")

        # ---- phase A: QKV projection ----
        qksb = ctx.enter_context(tc.tile_pool(name="qksb", bufs=1))
        # o-tiles: 0=qA(evens) 1=qB(odds) 2=kA 3=kB
        qkt = [qksb.tile([128, T], BF16, tag=f"qk{o}", name=f"qk{o}") for o in range(4)]
        stags = ("sA", "sB")
        utags = ("uA", "uB")
        for o in range(4):
            for c in range(4):
                pst = ps_tile([128, TC], stags[(o * 4 + c) % 2])
                for kk in range(4):
                    nc.tensor.matmul(pst[:],
                                     wqk8[:, 2 * kk:2 * kk + 2, ts(o, 128)],
                                     x8[:, 2 * kk:2 * kk + 2, ts(c, TC)],
                                     start=(kk == 0), stop=(kk == 3),
                                     perf_mode=DR)
                # bias add on ACT (idle during phase A); DVE is busy w/ rope
                nc.scalar.activation(qkt[o][:, ts(c, TC)], pst[:],
                                     AF.Identity, bias=qkb[:, o:o + 1])

        # v projection: [t, d'] layout, + ones column per head block
        vsb = ctx.enter_context(tc.tile_pool(name="vsb", bufs=1))
        vt = [vsb.tile([128, 260], BF16, tag=f"v{i}", name=f"v{i}") for i in range(NST)]
        v_pst = {}
        for i in range(NST):
            # v accumulates in the s-banks: the u-banks stay untouched by
            # phase A so the first pass's scores/exps never wait on v
            pst = ps_tile([128, 256], stags[i % 2])
            for j in range(8):
                nc.tensor.matmul(pst[:], xT[j][:, ts(i, 128)], wv[j][:],
                                 start=(j == 0), stop=(j == 7))
            v_pst[i] = pst

        def v_add(i):
            nc.vector.memset(vt[i][:], 1.0)
            # v block h at cols 65h:65h+64; col 65h+64 stays 1.0
            nc.vector.tensor_add(
                vt[i][:].rearrange("p (h d) -> p h d", h=4)[:, :, 0:64],
                v_pst[i][:].rearrange("p (h d) -> p h d", h=4),
                vbbc[:].rearrange("p (h d) -> p h d", h=4))

        # ---- rope: fp8 outputs in DoubleRow k-subtile layout ----
        # qEO/kEO [128, 2, T] fp8: [:,0,:] = rotated evens, [:,1,:] = odds
        ropet = ctx.enter_context(tc.tile_pool(name="ropet", bufs=2))
        rotsb = ctx.enter_context(tc.tile_pool(name="rotsb", bufs=1))
        qEO = rotsb.tile([128, 2, T], FP8, tag="qEO", name="qEO")
        kEO = rotsb.tile([128, 2, T], FP8, tag="kEO", name="kEO")

        def rope_chunk(which, c):
            a = qkt[0] if which == "q" else qkt[2]   # evens
            b = qkt[1] if which == "q" else qkt[3]   # odds
            eo = qEO if which == "q" else kEO
            cs = slice(c * TC, (c + 1) * TC)
            t1 = ropet.tile([128, TC], BF16, tag="t1")
            t2 = ropet.tile([128, TC], BF16, tag="t2")
            nc.vector.tensor_mul(t1[:], a[:, cs], cos4[:, cs])
            nc.vector.tensor_mul(t2[:], b[:, cs], sin4[:, cs])
            nc.vector.tensor_sub(eo[:, 0, cs], t1[:], t2[:])
            t3 = ropet.tile([128, TC], BF16, tag="t3")
            t4 = ropet.tile([128, TC], BF16, tag="t4")
            nc.vector.tensor_mul(t3[:], a[:, cs], sin4[:, cs])
            nc.vector.tensor_mul(t4[:], b[:, cs], cos4[:, cs])
            nc.vector.tensor_add(eo[:, 1, cs], t3[:], t4[:])

        # DVE order: rope chunks the first pass needs come first, v
        # bias-adds paced between the later chunks
        rope_chunk("q", 0)
        rope_chunk("q", 1)
        rope_chunk("k", 0)
        for i in (0, 1, 2, 3):
            v_add(i)
        rope_chunk("k", 1)
        for i in (4, 5, 6, 7):
            v_add(i)
        rope_chunk("k", 2)
        for i in (8, 9, 10, 11):
            v_add(i)
        rope_chunk("k", 3)
        for i in (12, 13, 14, 15):
            v_add(i)
        rope_chunk("q", 2)
        rope_chunk("q", 3)

        # ---- attention passes: (pair g, t-half th) ----
        ppool = ctx.enter_context(tc.tile_pool(name="pp", bufs=8))
        npool = ctx.enter_context(tc.tile_pool(name="np", bufs=2))
        unsb = ctx.enter_context(tc.tile_pool(name="unsb", bufs=1))
        ysb = ctx.enter_context(tc.tile_pool(name="ysb", bufs=3))

        # packed u_norm for the projection: [128 (pair c'), TH] per (g, th)
        upk = [[unsb.tile([128, TH], BF16, tag=f"upk{g}{th}", name=f"upk{g}{th}") for th in range(2)]
               for g in range(2)]

        def scores_exp(g, th, i):
            """Scores (fp8 DoubleRow, K=64 e/o packed) + exp for one s-tile;
            returns the two p tiles."""
            t0 = th * TH
            hA, hB = 2 * g, 2 * g + 1
            sps = {h: ps_tile([128, TH], utags[h - 2 * g])
                   for h in (hA, hB)}
            for c in range(2):
                for h in (hA, hB):
                    r = slice(32 * h, 32 * h + 32)
                    nc.tensor.matmul(
                        sps[h][:, ts(c, TC)],
                        kEO[r, :, ts(i, 128)],
                        qEO[r, :, t0 + c * TC:t0 + (c + 1) * TC],
                        start=True, stop=True, perf_mode=DR,
                        tile_position=(32 * h, 0))
            pd = {}
            for h in (hA, hB):
                p = ppool.tile([128, TH], BF16, tag=f"p{h - 2 * g}", name=f"p{h - 2 * g}")
                nc.scalar.activation(p[:], sps[h][:], AF.Exp, scale=0.125)
                pd[h] = p
            return pd

        def emit_pass(g, th, pre=None, nxt=None):
            """One (pair, t-half) attention pass.  `pre` carries p tiles
            prefetched by the previous pass; before our normalization chain
            we prefetch the first two score/exp groups of pass `nxt` so ACT
            never starves across the boundary."""
            hA, hB = 2 * g, 2 * g + 1
            u = {h: ps_tile([65, TH], stags[h - 2 * g]) for h in (hA, hB)}
            prefetched = None
            for i in range(NST):
                pd = pre[i] if (pre is not None and i < len(pre)) \
                    else scores_exp(g, th, i)
                for h in (hA, hB):
                    # AV + den: lhsT = [v_h | 1] (65 cols)
                    for c in range(2):
                        nc.tensor.matmul(
                            u[h][:, ts(c, TC)],
                            vt[i][:, 65 * h:65 * h + 65],
                            pd[h][:, ts(c, TC)],
                            start=(i == 0), stop=(i == NST - 1))
                if i == NST - 1 and nxt is not None:
                    prefetched = [scores_exp(nxt[0], nxt[1], 0),
                                  scores_exp(nxt[0], nxt[1], 1)]
            # normalization (per head): reciprocal of the denominator row,
            # ind-matmul broadcast to 64 partitions, then multiply — the
            # normalized attn out lands straight in the packed projection
            # tile (head A -> 0:64, B -> 64:128)
            for h in (hA, hB):
                dinv = npool.tile([65, TH], F32, tag="dinv", name="dinv")
                nc.vector.reciprocal(dinv[64:65, :], u[h][64:65, :])
                bc = ps_tile([128, TH], utags[h - 2 * g])
                for c in range(2):
                    nc.tensor.matmul(bc[:, ts(c, TC)],
                                     ind[64:65, :], dinv[64:65, ts(c, TC)],
                                     start=True, stop=True,
                                     tile_position=(64, 0))
                bcs = npool.tile([64, TH], F32, tag="bcs", name="bcs")
                nc.vector.tensor_copy(bcs[:], bc[0:64, :])
                nc.vector.tensor_mul(
                    upk[g][th][64 * (h % 2):64 * (h % 2) + 64, :],
                    u[h][0:64, :], bcs[:])
            return prefetched

        def proj_step(m, th, c, tag, idx):
            """One out-projection tile: yp = sum_g projw[g]^T @ upk[g][th].
            PSUM->SBUF copies alternate DVE/ACT, bf16 y DMAs alternate
            between the two hwdge queues."""
            yp = ps_tile([128, TC], tag)
            for g in range(2):
                nc.tensor.matmul(yp[:],
                                 projw[g][:, ts(m, 128)],
                                 upk[g][th][:, ts(c, TC)],
                                 start=(g == 0), stop=(g == 1))
            yt = ysb.tile([128, TC], BF16, tag="yt", name="yt")
            if idx % 2 == 0:
                nc.vector.tensor_copy(yt[:], yp[:])
            else:
                nc.scalar.copy(yt[:], yp[:])
            dma_eng = nc.sync if idx % 2 == 0 else nc.scalar
            dma_eng.dma_start(
                y_d[ts(m, 128), th * TH + c * TC:th * TH + (c + 1) * TC],
                yt[:])

        order = [(0, 0), (0, 1), (1, 0), (1, 1)]
        pre = None
        for n, (g, th) in enumerate(order):
            nxt = order[n + 1] if n + 1 < len(order) else None
            pre = emit_pass(g, th, pre=pre, nxt=nxt)
        for idx, (pm, pth, pc) in enumerate(
                [(m, th, c) for m in range(8) for th in range(2)
                 for c in range(2)]):
            proj_step(pm, pth, pc, stags[idx % 2], idx)


# ---------------- host-side prep / gather ----------------

def rope_tables():
    hd = HD
    inv_freq = 1.0 / (10000.0 ** (np.arange(0, hd, 2, dtype=np.float32) / hd))
    t = np.arange(T, dtype=np.float32)
    freqs = t[:, None] * inv_freq[None, :]                  # [T, 32]
    emb = np.concatenate([np.sin(freqs), np.cos(freqs)], axis=-1)  # [T,64]
    sin_t = emb[:, 0::2].T.astype(np.float32)               # [32, T]
    cos_t = emb[:, 1::2].T.astype(np.float32)
    return sin_t, cos_t


def make_in_maps(x, qkv_w, qkv_b, out_w):
    """Returns list of 8 per-core input dicts."""
    bf = ml_dtypes.bfloat16
    sin_t, cos_t = rope_tables()
    cos4 = np.tile(cos_t, (4, 1)).astype(bf)
    sin4 = np.tile(sin_t, (4, 1)).astype(bf)
    ind = np.zeros((128, 128), np.float32)
    ind[64, 0:64] = 1.0
    ev = np.arange(0, HD, 2)
    od = np.arange(1, HD, 2)

    in_maps = []
    for core in range(8):
        b = core // 4
        h0 = HPC * (core % 4)
        heads = np.arange(h0, h0 + HPC)
        qA = np.concatenate([h * HD + ev for h in heads])          # 128
        qB = np.concatenate([h * HD + od for h in heads])
        kA = DIM + qA
        kB = DIM + qB
        qk_rows = np.concatenate([qA, qB, kA, kB])                  # 512
        v_rows = 2 * DIM + np.arange(h0 * HD, (h0 + HPC) * HD)      # 256
        f8 = ml_dtypes.float8_e4m3
        wqk8 = np.ascontiguousarray(
            qkv_w[qk_rows, :].T.reshape(8, 128, 512).transpose(1, 0, 2)
        ).astype(f8)                                                 # [128,8,512]
        qkb = np.ascontiguousarray(
            qkv_b[qk_rows].reshape(4, 128).T).astype(np.float32)     # [128,4]
        wvT = np.ascontiguousarray(qkv_w[v_rows, :].T).astype(bf)    # [1024,256]
        vb_bc = np.broadcast_to(qkv_b[v_rows], (128, 256)).astype(np.float32)
        projwT = np.ascontiguousarray(
            out_w[:, h0 * HD:(h0 + HPC) * HD].T).astype(bf)          # [256,1024]
        xT = np.ascontiguousarray(x[b].T).astype(bf)                 # [1024,2048]
        x8 = np.ascontiguousarray(
            x[b].T.reshape(8, 128, T).transpose(1, 0, 2)).astype(f8)  # [128,8,T]
        in_maps.append({
            "xT": np.asarray(xT), "x8": np.asarray(x8),
            "wqk8": np.asarray(wqk8), "qkb": qkb,
            "wvT": np.asarray(wvT), "vb_bc": np.ascontiguousarray(vb_bc),
            "cos4": np.asarray(cos4), "sin4": np.asarray(sin4),
            "ind": ind, "projwT": np.asarray(projwT),
        })
    return in_maps


def gather(results, out_b):
    """results: list of 8 dicts with y [1024, 2048] f32 partials."""
    y = np.zeros((B, T, DIM), np.float32)
    for core in range(8):
        b = core // 4
        y[b] += results[core]["y"].T.astype(np.float32)
    y += out_b[None, None, :]
    return y


# ---------------- harness entry point ----------------

_NC_CACHE = {}


def kernel(x, qkv_w, qkv_b, out_w, out_b):
    """Full-input entry: shards across 8 NeuronCores, returns full output."""
    from concourse import bass_utils
    x = np.asarray(x); qkv_w = np.asarray(qkv_w); qkv_b = np.asarray(qkv_b)
    out_w = np.asarray(out_w); out_b = np.asarray(out_b)
    if "nc" not in _NC_CACHE:
        _NC_CACHE["nc"] = build(n_cores=8)
    nc = _NC_CACHE["nc"]
    in_maps = make_in_maps(x, qkv_w, qkv_b, out_w)
    res = bass_utils.run_bass_kernel_spmd(nc, in_maps, core_ids=list(range(8)))
    return gather(res.results, out_b)


# revision 49
# speedup vs baseline: 1.3802x; 1.1728x over previous
"""MultiHeadAttention Bass kernel for TRN2, 8-core SPMD.

Sharding: core c -> batch b = c//4, heads [4*(c%4), 4*(c%4)+4).
Each core computes the qkv projection for its 4 heads, rope, attention,
and the out-projection partial (its 256 channels); host sums partials.

Device tensors (per core, host-prepped):
  xT        [1024, 2048]  bf16   x[b].T (channels on partitions; v-proj)
  x8        [128, 8, 2048] fp8e4 x[b].T as 8 k-tiles on dim 1 (q/k-proj)
  wqk8      [128, 8, 512] fp8e4  q/k weight cols perm [qA|qB|kA|kB],
                                 qA = even d-idx of 4 heads (4x32), qB = odds
  qkb       [128, 4]      f32    per-partition bias for the 4 o-tiles
  wvT       [1024, 256]   bf16   v weights, natural order
  vb_bc     [128, 256]    f32    v bias broadcast across partitions
  cos4/sin4 [128, 2048]   bf16   rope tables tiled 4x heads
  ind       [128, 128]    f32    row 64, cols 0:64 = 1 (recip broadcast matmul)
  projwT    [256, 1024]   bf16   out_w cols for this core's heads, transposed
  y         [1024, 2048]  bf16   OUT: partial y^T (m on rows); host upcasts

Key performance structure (For_i repeats run back-to-back behind an
all-engine barrier, so the single-iteration critical path is what counts):
 - q/k projection and scores run as fp8e4 DoubleRow matmuls (2 k-subtiles
   per pass, 0.5 cyc/col); scores pack K=64 = even|odd rope halves from
   [128, 2, T] fp8 rope outputs written directly by DVE.
 - ACT is the bottleneck engine (128 exps of [128,1024]); everything else
   is scheduled around keeping it fed: critical DMAs (wqk8, x8, rope
   tables) issue first; rope is emitted in 512-col chunks interleaved
   with the v bias-adds so the first exp fires ~12us in; each pass
   prefetches the next pass's first two score/exp groups before its own
   normalization chain so ACT never starves at pass boundaries.
 - PSUM tag roles: phase A + u-accumulators on sA/sB, scores/bc on uA/uB
   (so the exp stream never waits on the v pipeline); deep p-pool lets
   AV lag behind exp.
 - tail: out-projection drains on the s-banks; PSUM->SBUF copies
   alternate DVE/ACT and bf16 y DMAs alternate across both hwdge queues.
"""

import numpy as np
import ml_dtypes

import concourse.bass as bass
import concourse.tile as tile
from concourse import bacc, mybir
from concourse.bass import ts

F32 = mybir.dt.float32
BF16 = mybir.dt.bfloat16
FP8 = mybir.dt.float8e4
AF = mybir.ActivationFunctionType
DR = mybir.MatmulPerfMode.DoubleRow

B, T, DIM, NH = 2, 2048, 1024, 16
HD = 64          # head dim
HPC = 4          # heads per core
TC = 512         # t-chunk (one psum bank / fp32 matmul N limit)
TH = 1024        # t-half (exp op size)
NST = T // 128   # 16 s-tiles


def build(n_cores=8, loop_reps=1):
    nc = bacc.Bacc("TRN2", target_bir_lowering=False, debug=False,
                   num_devices=n_cores)

    xT_d = nc.dram_tensor("xT", [DIM, T], BF16, kind="ExternalInput").ap()
    x8_d = nc.dram_tensor("x8", [128, 8, T], FP8, kind="ExternalInput").ap()
    wqk8_d = nc.dram_tensor("wqk8", [128, 8, 512], FP8, kind="ExternalInput").ap()
    qkb_d = nc.dram_tensor("qkb", [128, 4], F32, kind="ExternalInput").ap()
    wvT_d = nc.dram_tensor("wvT", [DIM, 256], BF16, kind="ExternalInput").ap()
    vbbc_d = nc.dram_tensor("vb_bc", [128, 256], F32, kind="ExternalInput").ap()
    cos4_d = nc.dram_tensor("cos4", [128, T], BF16, kind="ExternalInput").ap()
    sin4_d = nc.dram_tensor("sin4", [128, T], BF16, kind="ExternalInput").ap()
    ind_d = nc.dram_tensor("ind", [128, 128], F32, kind="ExternalInput").ap()
    projwT_d = nc.dram_tensor("projwT", [256, 1024], BF16, kind="ExternalInput").ap()
    y_d = nc.dram_tensor("y", [DIM, T], BF16, kind="ExternalOutput").ap()

    with tile.TileContext(nc) as tc:
        if loop_reps > 1:
            with tc.For_i(0, loop_reps, 1):
                _kernel(nc, tc, xT_d, x8_d, wqk8_d, qkb_d, wvT_d, vbbc_d,
                        cos4_d, sin4_d, ind_d, projwT_d, y_d)
        else:
            _kernel(nc, tc, xT_d, x8_d, wqk8_d, qkb_d, wvT_d, vbbc_d,
                    cos4_d, sin4_d, ind_d, projwT_d, y_d)
    nc.compile()
    return nc


def _kernel(nc, tc, xT_d, x8_d, wqk8_d, qkb_d, wvT_d, vbbc_d, cos4_d, sin4_d,
            ind_d, projwT_d, y_d):
    from contextlib import ExitStack
    ctx = ExitStack()
    with ctx:
        # ---- constant / weight pools ----
        consts = ctx.enter_context(tc.tile_pool(name="consts", bufs=1))
        xpool = ctx.enter_context(tc.tile_pool(name="xp", bufs=1))

        # fp8 x / qk weights for the DoubleRow q/k projection go FIRST —
        # they gate the very first matmuls.  [128, j(8), cols], k-tile on dim 1
        wqk8 = consts.tile([128, 8, 512], FP8, tag="wqk8", name="wqk8")
        nc.sync.dma_start(wqk8[:], wqk8_d[:])
        x8 = xpool.tile([128, 8, T], FP8, tag="x8", name="x8")
        nc.sync.dma_start(x8[:], x8_d[:])
        qkb = consts.tile([128, 4], F32, tag="qkb")
        nc.sync.dma_start(qkb[:], qkb_d[:])
        cos4 = consts.tile([128, T], BF16, tag="cos4")
        nc.sync.dma_start(cos4[:], cos4_d[:])
        sin4 = consts.tile([128, T], BF16, tag="sin4")
        nc.sync.dma_start(sin4[:], sin4_d[:])
        xT = [xpool.tile([128, T], BF16, tag=f"xT{j}", name=f"xT{j}") for j in range(8)]
        for j in range(8):
            nc.sync.dma_start(xT[j][:], xT_d[ts(j, 128), :])
        wv = [consts.tile([128, 256], BF16, tag=f"wv{j}", name=f"wv{j}") for j in range(8)]
        for j in range(8):
            nc.sync.dma_start(wv[j][:], wvT_d[ts(j, 128), :])
        vbbc = consts.tile([128, 256], F32, tag="vbbc")
        nc.sync.dma_start(vbbc[:], vbbc_d[:])
        ind = consts.tile([128, 128], F32, tag="ind")
        nc.sync.dma_start(ind[:], ind_d[:])
        projw = [consts.tile([128, 1024], BF16, tag=f"pw{g}", name=f"pw{g}") for g in range(2)]
        for g in range(2):
            nc.sync.dma_start(projw[g][:], projwT_d[ts(g, 128), :])

        # ---- single shared PSUM pool: 4 tags x 2 banks = 8 banks ----
        ps = ctx.enter_context(tc.tile_pool(name="ps", bufs=1, space="PSUM"))

        def ps_tile(shape, tag):
            return ps.tile(shape, F32, tag=tag, name=f"ps_{tag}")

        # ---- phase A: QKV projection ----
        qksb = ctx.enter_context(tc.tile_pool(name="qksb", bufs=1))
        # o-tiles: 0=qA(evens) 1=qB(odds) 2=kA 3=kB
        qkt = [qksb.tile([128, T], BF16, tag=f"qk{o}", name=f"qk{o}") for o in range(4)]
        stags = ("sA", "sB")
        utags = ("uA", "uB")
        for o in range(4):
            for c in range(4):
                pst = ps_tile([128, TC], stags[(o * 4 + c) % 2])
                for kk in range(4):
                    nc.tensor.matmul(pst[:],
                                     wqk8[:, 2 * kk:2 * kk + 2, ts(o, 128)],
                                     x8[:, 2 * kk:2 * kk + 2, ts(c, TC)],
                                     start=(kk == 0), stop=(kk == 3),
                                     perf_mode=DR)
                # bias add on ACT (idle during phase A); DVE is busy w/ rope
                nc.scalar.activation(qkt[o][:, ts(c, TC)], pst[:],
                                     AF.Identity, bias=qkb[:, o:o + 1])

        # v projection: [t, d'] layout, + ones column per head block
        vsb = ctx.enter_context(tc.tile_pool(name="vsb", bufs=1))
        vt = [vsb.tile([128, 260], BF16, tag=f"v{i}", name=f"v{i}") for i in range(NST)]
        v_pst = {}
        for i in range(NST):
            # v accumulates in the s-banks: the u-banks stay untouched by
            # phase A so the first pass's scores/exps never wait on v
            pst = ps_tile([128, 256], stags[i % 2])
            for j in range(8):
                nc.tensor.matmul(pst[:], xT[j][:, ts(i, 128)], wv[j][:],
                                 start=(j == 0), stop=(j == 7))
            v_pst[i] = pst

        def v_add(i):
            nc.vector.memset(vt[i][:], 1.0)
            # v block h at cols 65h:65h+64; col 65h+64 stays 1.0
            nc.vector.tensor_add(
                vt[i][:].rearrange("p (h d) -> p h d", h=4)[:, :, 0:64],
                v_pst[i][:].rearrange("p (h d) -> p h d", h=4),
                vbbc[:].rearrange("p (h d) -> p h d", h=4))

        # ---- rope: fp8 outputs in DoubleRow k-subtile layout ----
        # qEO/kEO [128, 2, T] fp8: [:,0,:] = rotated evens, [:,1,:] = odds
        ropet = ctx.enter_context(tc.tile_pool(name="ropet", bufs=2))
        rotsb = ctx.enter_context(tc.tile_pool(name="rotsb", bufs=1))
        qEO = rotsb.tile([128, 2, T], FP8, tag="qEO", name="qEO")
        kEO = rotsb.tile([128, 2, T], FP8, tag="kEO", name="kEO")

        def rope_chunk(which, c):
            a = qkt[0] if which == "q" else qkt[2]   # evens
            b = qkt[1] if which == "q" else qkt[3]   # odds
            eo = qEO if which == "q" else kEO
            cs = slice(c * TC, (c + 1) * TC)
            t1 = ropet.tile([128, TC], BF16, tag="t1")
            t2 = ropet.tile([128, TC], BF16, tag="t2")
            nc.vector.tensor_mul(t1[:], a[:, cs], cos4[:, cs])
            nc.vector.tensor_mul(t2[:], b[:, cs], sin4[:, cs])
            nc.vector.tensor_sub(eo[:, 0, cs], t1[:], t2[:])
            t3 = ropet.tile([128, TC], BF16, tag="t3")
            t4 = ropet.tile([128, TC], BF16, tag="t4")
            nc.vector.tensor_mul(t3[:], a[:, cs], sin4[:, cs])
            nc.vector.tensor_mul(t4[:], b[:, cs], cos4[:, cs])
            nc.vector.tensor_add(eo[:, 1, cs], t3[:], t4[:])

        # DVE order: rope chunks the first pass needs come first, v
        # bias-adds paced between the later chunks
        rope_chunk("q", 0)
        rope_chunk("q", 1)
        rope_chunk("k", 0)
        for i in (0, 1, 2, 3):
            v_add(i)
        rope_chunk("k", 1)
        for i in (4, 5, 6, 7):
            v_add(i)
        rope_chunk("k", 2)
        for i in (8, 9, 10, 11):
            v_add(i)
        rope_chunk("k", 3)
        for i in (12, 13, 14, 15):
            v_add(i)
        rope_chunk("q", 2)
        rope_chunk("q", 3)

        # ---- attention passes: (pair g, t-half th) ----
        ppool = ctx.enter_context(tc.tile_pool(name="pp", bufs=8))
        npool = ctx.enter_context(tc.tile_pool(name="np", bufs=3))
        unsb = ctx.enter_context(tc.tile_pool(name="unsb", bufs=1))
        ysb = ctx.enter_context(tc.tile_pool(name="ysb", bufs=6))

        # packed u_norm for the projection: [128 (pair c'), TH] per (g, th)
        upk = [[unsb.tile([128, TH], BF16, tag=f"upk{g}{th}", name=f"upk{g}{th}") for th in range(2)]
               for g in range(2)]

        def scores_exp(g, th, i):
            """Scores (fp8 DoubleRow, K=64 e/o packed) + exp for one s-tile;
            returns the two p tiles."""
            t0 = th * TH
            hA, hB = 2 * g, 2 * g + 1
            sps = {h: ps_tile([128, TH], utags[h - 2 * g])
                   for h in (hA, hB)}
            for c in range(2):
                for h in (hA, hB):
                    r = slice(32 * h, 32 * h + 32)
                    nc.tensor.matmul(
                        sps[h][:, ts(c, TC)],
                        kEO[r, :, ts(i, 128)],
                        qEO[r, :, t0 + c * TC:t0 + (c + 1) * TC],
                        start=True, stop=True, perf_mode=DR,
                        tile_position=(32 * h, 0))
            pd = {}
            for h in (hA, hB):
                p = ppool.tile([128, TH], BF16, tag=f"p{h - 2 * g}", name=f"p{h - 2 * g}")
                nc.scalar.activation(p[:], sps[h][:], AF.Exp, scale=0.125)
                pd[h] = p
            return pd

        def emit_pass(g, th, pre=None, nxt=None):
            """One (pair, t-half) attention pass.  `pre` carries p tiles
            prefetched by the previous pass; before our normalization chain
            we prefetch the first two score/exp groups of pass `nxt` so ACT
            never starves across the boundary."""
            hA, hB = 2 * g, 2 * g + 1
            u = {h: ps_tile([65, TH], stags[h - 2 * g]) for h in (hA, hB)}
            prefetched = None
            for i in range(NST):
                pd = pre[i] if (pre is not None and i < len(pre)) \
                    else scores_exp(g, th, i)
                for h in (hA, hB):
                    # AV + den: lhsT = [v_h | 1] (65 cols)
                    for c in range(2):
                        nc.tensor.matmul(
                            u[h][:, ts(c, TC)],
                            vt[i][:, 65 * h:65 * h + 65],
                            pd[h][:, ts(c, TC)],
                            start=(i == 0), stop=(i == NST - 1))
                if i == NST - 1 and nxt is not None:
                    prefetched = [scores_exp(nxt[0], nxt[1], 0),
                                  scores_exp(nxt[0], nxt[1], 1),
                                  scores_exp(nxt[0], nxt[1], 2)]
            # normalization (per head): reciprocal of the denominator row,
            # ind-matmul broadcast to 64 partitions, then multiply — the
            # normalized attn out lands straight in the packed projection
            # tile (head A -> 0:64, B -> 64:128)
            for h in (hA, hB):
                dinv = npool.tile([65, TH], F32, tag="dinv", name="dinv")
                nc.vector.reciprocal(dinv[64:65, :], u[h][64:65, :])
                bc = ps_tile([128, TH], utags[h - 2 * g])
                for c in range(2):
                    nc.tensor.matmul(bc[:, ts(c, TC)],
                                     ind[64:65, :], dinv[64:65, ts(c, TC)],
                                     start=True, stop=True,
                                     tile_position=(64, 0))
                bcs = npool.tile([64, TH], F32, tag="bcs", name="bcs")
                nc.vector.tensor_copy(bcs[:], bc[0:64, :])
                nc.vector.tensor_mul(
                    upk[g][th][64 * (h % 2):64 * (h % 2) + 64, :],
                    u[h][0:64, :], bcs[:])
            return prefetched

        def proj_step(m, th, c, tag, idx):
            """One out-projection tile: yp = sum_g projw[g]^T @ upk[g][th].
            PSUM->SBUF copies alternate DVE/ACT, bf16 y DMAs alternate
            between the two hwdge queues."""
            yp = ps_tile([128, TC], tag)
            for g in range(2):
                nc.tensor.matmul(yp[:],
                                 projw[g][:, ts(m, 128)],
                                 upk[g][th][:, ts(c, TC)],
                                 start=(g == 0), stop=(g == 1))
            yt = ysb.tile([128, TC], BF16, tag="yt", name="yt")
            if idx % 2 == 0:
                nc.vector.tensor_copy(yt[:], yp[:])
            else:
                nc.scalar.copy(yt[:], yp[:])
            dma_eng = nc.sync if idx % 2 == 0 else nc.scalar
            dma_eng.dma_start(
                y_d[ts(m, 128), th * TH + c * TC:th * TH + (c + 1) * TC],
                yt[:])

        order = [(0, 0), (0, 1), (1, 0), (1, 1)]
        pre = None
        for n, (g, th) in enumerate(order):
            nxt = order[n + 1] if n + 1 < len(order) else None
            pre = emit_pass(g, th, pre=pre, nxt=nxt)
        for idx, (pm, pth, pc) in enumerate(
                [(m, th, c) for m in range(8) for th in range(2)
                 for c in range(2)]):
            proj_step(pm, pth, pc, stags[idx % 2], idx)


# ---------------- host-side prep / gather ----------------

def rope_tables():
    hd = HD
    inv_freq = 1.0 / (10000.0 ** (np.arange(0, hd, 2, dtype=np.float32) / hd))
    t = np.arange(T, dtype=np.float32)
    freqs = t[:, None] * inv_freq[None, :]                  # [T, 32]
    emb = np.concatenate([np.sin(freqs), np.cos(freqs)], axis=-1)  # [T,64]
    sin_t = emb[:, 0::2].T.astype(np.float32)               # [32, T]
    cos_t = emb[:, 1::2].T.astype(np.float32)
    return sin_t, cos_t


def make_in_maps(x, qkv_w, qkv_b, out_w):
    """Returns list of 8 per-core input dicts."""
    bf = ml_dtypes.bfloat16
    sin_t, cos_t = rope_tables()
    cos4 = np.tile(cos_t, (4, 1)).astype(bf)
    sin4 = np.tile(sin_t, (4, 1)).astype(bf)
    ind = np.zeros((128, 128), np.float32)
    ind[64, 0:64] = 1.0
    ev = np.arange(0, HD, 2)
    od = np.arange(1, HD, 2)

    in_maps = []
    for core in range(8):
        b = core // 4
        h0 = HPC * (core % 4)
        heads = np.arange(h0, h0 + HPC)
        qA = np.concatenate([h * HD + ev for h in heads])          # 128
        qB = np.concatenate([h * HD + od for h in heads])
        kA = DIM + qA
        kB = DIM + qB
        qk_rows = np.concatenate([qA, qB, kA, kB])                  # 512
        v_rows = 2 * DIM + np.arange(h0 * HD, (h0 + HPC) * HD)      # 256
        f8 = ml_dtypes.float8_e4m3
        wqk8 = np.ascontiguousarray(
            qkv_w[qk_rows, :].T.reshape(8, 128, 512).transpose(1, 0, 2)
        ).astype(f8)                                                 # [128,8,512]
        qkb = np.ascontiguousarray(
            qkv_b[qk_rows].reshape(4, 128).T).astype(np.float32)     # [128,4]
        wvT = np.ascontiguousarray(qkv_w[v_rows, :].T).astype(bf)    # [1024,256]
        vb_bc = np.broadcast_to(qkv_b[v_rows], (128, 256)).astype(np.float32)
        projwT = np.ascontiguousarray(
            out_w[:, h0 * HD:(h0 + HPC) * HD].T).astype(bf)          # [256,1024]
        xT = np.ascontiguousarray(x[b].T).astype(bf)                 # [1024,2048]
        x8 = np.ascontiguousarray(
            x[b].T.reshape(8, 128, T).transpose(1, 0, 2)).astype(f8)  # [128,8,T]
        in_maps.append({
            "xT": np.asarray(xT), "x8": np.asarray(x8),
            "wqk8": np.asarray(wqk8), "qkb": qkb,
            "wvT": np.asarray(wvT), "vb_bc": np.ascontiguousarray(vb_bc),
            "cos4": np.asarray(cos4), "sin4": np.asarray(sin4),
            "ind": ind, "projwT": np.asarray(projwT),
        })
    return in_maps


def gather(results, out_b):
    """results: list of 8 dicts with y [1024, 2048] f32 partials."""
    y = np.zeros((B, T, DIM), np.float32)
    for core in range(8):
        b = core // 4
        y[b] += results[core]["y"].T.astype(np.float32)
    y += out_b[None, None, :]
    return y


# ---------------- harness entry point ----------------

_NC_CACHE = {}


def kernel(x, qkv_w, qkv_b, out_w, out_b):
    """Full-input entry: shards across 8 NeuronCores, returns full output."""
    from concourse import bass_utils
    x = np.asarray(x); qkv_w = np.asarray(qkv_w); qkv_b = np.asarray(qkv_b)
    out_w = np.asarray(out_w); out_b = np.asarray(out_b)
    if "nc" not in _NC_CACHE:
        _NC_CACHE["nc"] = build(n_cores=8)
    nc = _NC_CACHE["nc"]
    in_maps = make_in_maps(x, qkv_w, qkv_b, out_w)
    res = bass_utils.run_bass_kernel_spmd(nc, in_maps, core_ids=list(range(8)))
    return gather(res.results, out_b)
